# revision 13
# baseline (speedup 1.0000x reference)
"""Classical single-head self-attention on 8 Trainium2 NeuronCores.

Problem (hardcoded): x [4, 2048, 1024] f32, Wq/Wk/Wv [1024, 1024] f32.
    q = x @ Wq.T ; k = x @ Wk.T ; v = x @ Wv.T
    out = softmax(q @ k.T / sqrt(1024)) @ v

Sharding: 8 cores = 4 batches x 2 query-halves. Core c handles batch
c//2 and queries [h*1024, (h+1)*1024) with h = c%2. Each core computes
k/v for its batch's full 2048-token sequence (duplicated across the
pair of cores sharing a batch); no collectives.

Host-side layout trick: each core receives xT = x[b].T (shape [1024 d,
2048 s]) with the sequence axis rolled so the core's own query half
always occupies columns [0, 1024). Attention is permutation-invariant
over keys, so rolling the key order changes nothing. Weights are passed
pre-transposed ([d, e]) and Wq is pre-scaled by 1/sqrt(d), so every
matmul has its contraction dim on SBUF partitions.

On-core pipeline (all matmuls fp32r except the attn@v stage in bf16):
  1. qT [e, sq=1024] = WqT.T @ xT[:, :1024]          (resident f32)
  2. kT [e, sk=2048] -> bounced to DRAM scratch      (SBUF too small)
  3. v  [sk, e]      = xT.T @ WvT                    (resident bf16)
  4. per sq-chunk of 512: scoresT [sk, sq'] = kT.T @ qT', exp via ACT
     (no max subtraction: scores are O(5) by construction),
     row-sums via ones-vector matmul, out = expT.T @ v scaled by 1/sum.
"""

import numpy as np
from contextlib import ExitStack

import concourse.bacc as bacc
import concourse.tile as tile
from concourse import mybir
from concourse.bass_utils import run_bass_kernel_spmd

F32 = mybir.dt.float32
F32R = mybir.dt.float32r
BF16 = mybir.dt.bfloat16
EXP = mybir.ActivationFunctionType.Exp

N_CORES = 8



def build_nc(D=1024, S=2048, SQ=1024, av_bf16=True, debug_skip=()):
    """Emit the per-core kernel. D = embed dim, S = keys, SQ = queries."""
    P = 128
    DT = D // P          # d tiles (contraction for projections)
    ET = D // P          # e tiles
    KT = S // P          # sk tiles
    NCH = SQ // 512      # sq chunks of 512
    ECH = D // 512       # e chunks of 512

    nc = bacc.Bacc("TRN2", target_bir_lowering=False)

    xT_d = nc.dram_tensor("xT", [D, S], F32, kind="ExternalInput")
    wqT_d = nc.dram_tensor("wqT", [D, D], F32, kind="ExternalInput")
    wkT_d = nc.dram_tensor("wkT", [D, D], F32, kind="ExternalInput")
    wvT_d = nc.dram_tensor("wvT", [D, D], F32, kind="ExternalInput")
    out_d = nc.dram_tensor("out", [SQ, D], F32, kind="ExternalOutput")

    av_t = BF16 if av_bf16 else F32

    _emit_body(nc, locals())
    nc.compile()
    return nc


def _emit_body(nc, cfg):
    P, DT, ET, KT, NCH, ECH, S, SQ, D = (cfg[k] for k in
        ("P", "DT", "ET", "KT", "NCH", "ECH", "S", "SQ", "D"))
    av_t, debug_skip = cfg["av_t"], cfg["debug_skip"]
    xT_d, wqT_d, wkT_d, wvT_d, out_d = (cfg[k] for k in
        ("xT_d", "wqT_d", "wkT_d", "wvT_d", "out_d"))
    with ExitStack() as ctx:
        tc = ctx.enter_context(tile.TileContext(nc))
        dram = ctx.enter_context(tc.tile_pool(name="dram", bufs=1, space="DRAM"))
        kT_dram = dram.tile([D, S], F32R)
        sums_dram = dram.tile([NCH, 512], F32)

        qt_pool = ctx.enter_context(tc.tile_pool(name="qt", bufs=1))
        v_pool = ctx.enter_context(tc.tile_pool(name="v", bufs=1))
        misc = ctx.enter_context(tc.tile_pool(name="misc", bufs=1))
        mm_ps = ctx.enter_context(tc.tile_pool(name="mmps", bufs=4, space="PSUM"))

        qt = qt_pool.tile([P, ET, SQ], F32R)
        v = v_pool.tile([P, KT, D], av_t)
        ones = misc.tile([P, 1], av_t)
        nc.vector.memset(ones, 1.0)

        # ---------------- projections ----------------
        with (
            tc.tile_pool(name="xt", bufs=1) as xt_pool,
            tc.tile_pool(name="w", bufs=12) as w_pool,
            tc.tile_pool(name="ktsb", bufs=3) as ktsb_pool,
        ):
            xt = xt_pool.tile([P, DT, S], F32R)
            nc.sync.dma_start(
                out=xt, in_=xT_d[:].rearrange("(dt p) s -> p dt s", p=P).bitcast(F32R)
            )

            def load_w(w_dram):
                tiles = []
                for dt_i in range(DT):
                    w_t = w_pool.tile([P, D], F32R, tag="w")
                    nc.sync.dma_start(
                        out=w_t, in_=w_dram[dt_i * P : (dt_i + 1) * P, :].bitcast(F32R)
                    )
                    tiles.append(w_t)
                return tiles

            wq = load_w(wqT_d)
            wk = load_w(wkT_d)
            wv = load_w(wvT_d)

            # qT[e, sq]: lhsT = wqT[d, e-tile], rhs = xT[d, sq-chunk]
            for et in range(ET):
                for chk in range(NCH):
                    ps = mm_ps.tile([P, 512], F32, tag="mm")
                    for dt_i in range(DT):
                        nc.tensor.matmul(
                            ps,
                            wq[dt_i][:, et * P : (et + 1) * P],
                            xt[:, dt_i, chk * 512 : (chk + 1) * 512],
                            start=(dt_i == 0),
                            stop=(dt_i == DT - 1),
                        )
                    nc.vector.tensor_copy(qt[:, et, chk * 512 : (chk + 1) * 512], ps)

            # kT[e, sk] -> DRAM scratch
            for et in range(ET):
                for sc in range(S // 512):
                    ps = mm_ps.tile([P, 512], F32, tag="mm")
                    for dt_i in range(DT):
                        nc.tensor.matmul(
                            ps,
                            wk[dt_i][:, et * P : (et + 1) * P],
                            xt[:, dt_i, sc * 512 : (sc + 1) * 512],
                            start=(dt_i == 0),
                            stop=(dt_i == DT - 1),
                        )
                    kt_sb = ktsb_pool.tile([P, 512], F32R, tag="ktsb")
                    nc.vector.tensor_copy(kt_sb, ps)
                    nc.sync.dma_start(
                        out=kT_dram[et * P : (et + 1) * P, sc * 512 : (sc + 1) * 512],
                        in_=kt_sb,
                    )

            # v[sk, e]: lhsT = xT[d, sk-tile], rhs = wvT[d, e-chunk]
            for kt_i in range(KT):
                for ec in range(ECH):
                    ps = mm_ps.tile([P, 512], F32, tag="mm")
                    for dt_i in range(DT):
                        nc.tensor.matmul(
                            ps,
                            xt[:, dt_i, kt_i * P : (kt_i + 1) * P],
                            wv[dt_i][:, ec * 512 : (ec + 1) * 512],
                            start=(dt_i == 0),
                            stop=(dt_i == DT - 1),
                        )
                    nc.scalar.copy(v[:, kt_i, ec * 512 : (ec + 1) * 512], ps)

        # ---------------- attention ----------------
        if "attn" in debug_skip:
            dummy = qt_pool.tile([P, 512], F32)
            nc.vector.tensor_copy(dummy, qt[:, 0, 0:512])
            nc.sync.dma_start(out=out_d[0:P, 0:512], in_=dummy)
            return
        with (
            tc.tile_pool(name="ktin", bufs=4) as ktin_pool,
            tc.tile_pool(name="expt", bufs=2) as expt_pool,
            tc.tile_pool(name="osb", bufs=3) as osb_pool,
            tc.tile_pool(name="sums", bufs=2) as sums_pool,
            tc.tile_pool(name="scps", bufs=3, space="PSUM") as sc_ps,
            tc.tile_pool(name="smps", bufs=1, space="PSUM") as sm_ps,
        ):
            kT_r = kT_dram[:].rearrange("(et p) s -> p et s", p=P)
            for chk in range(NCH):
                sq_lo = chk * 512
                expt = expt_pool.tile([P, KT, 512], av_t, tag="expt")
                if "sums" not in debug_skip:
                    sums_ps = sm_ps.tile([1, 512], F32, tag="sums", name="sums_ps")
                else:
                    sums_ps = None
                for kt_i in range(KT):
                    ktt = ktin_pool.tile([P, ET, P], F32R, tag="ktin")
                    nc.sync.dma_start(
                        out=ktt, in_=kT_r[:, :, kt_i * P : (kt_i + 1) * P]
                    )
                    ps = sc_ps.tile([P, 512], F32, tag="sc")
                    for et in range(ET):
                        nc.tensor.matmul(
                            ps,
                            ktt[:, et, :],
                            qt[:, et, sq_lo : sq_lo + 512],
                            start=(et == 0),
                            stop=(et == ET - 1),
                        )
                    if "exp" in debug_skip:
                        nc.scalar.copy(expt[:, kt_i, :], ps)
                    else:
                        nc.scalar.activation(expt[:, kt_i, :], ps, EXP)
                    if "sums" not in debug_skip:
                        nc.tensor.matmul(
                            sums_ps,
                            ones,
                            expt[:, kt_i, :],
                            start=(kt_i == 0),
                            stop=(kt_i == KT - 1),
                        )

                # sums [1, 512] -> recipT [128, 4] via DRAM bounce
                recipT = sums_pool.tile([P, 4], F32, tag="recipT")
                if "sums" in debug_skip:
                    nc.vector.memset(recipT, 1.0)
                else:
                    sums_sb = sums_pool.tile([1, 512], F32, tag="sums_sb")
                    nc.scalar.copy(sums_sb, sums_ps)
                    nc.sync.dma_start(out=sums_dram[chk], in_=sums_sb[:])
                    nc.sync.dma_start(
                        out=recipT,
                        in_=sums_dram[chk].rearrange("(j p) -> p j", p=P),
                    )
                    nc.vector.reciprocal(recipT, recipT)

                # out[sq, e] = expT.T @ v, scaled by 1/rowsum
                for st in range(4):
                    for ec in range(ECH):
                        ps = mm_ps.tile([P, 512], F32, tag="mm")
                        for kt_i in range(KT):
                            nc.tensor.matmul(
                                ps,
                                expt[:, kt_i, st * P : (st + 1) * P],
                                v[:, kt_i, ec * 512 : (ec + 1) * 512],
                                start=(kt_i == 0),
                                stop=(kt_i == KT - 1),
                            )
                        osb = osb_pool.tile([P, 512], F32, tag="osb")
                        nc.vector.tensor_scalar_mul(osb, ps, recipT[:, st : st + 1])
                        row = sq_lo + st * P
                        nc.sync.dma_start(
                            out=out_d[row : row + P, ec * 512 : (ec + 1) * 512],
                            in_=osb,
                        )


_NC_CACHE = {}


def _get_nc():
    if "nc" not in _NC_CACHE:
        _NC_CACHE["nc"] = build_nc()
    return _NC_CACHE["nc"]


def _get_sharded_fn():
    """jit-once 8-core executor mirroring bass2jax.run_bass_via_pjrt."""
    if "fn" in _NC_CACHE:
        return _NC_CACHE["fn"]
    import jax
    from jax.experimental.shard_map import shard_map
    from jax.sharding import Mesh, PartitionSpec
    from concourse import mybir as _mybir
    from concourse import bass2jax

    nc = _get_nc()
    bass2jax.install_neuronx_cc_hook()
    partition_name = nc.partition_id_tensor.name if nc.partition_id_tensor else None
    in_names, out_names, out_avals, zero_outs = [], [], [], []
    for alloc in nc.m.functions[0].allocations:
        if not isinstance(alloc, _mybir.MemoryLocationSet):
            continue
        name = alloc.memorylocations[0].name
        if alloc.kind == "ExternalInput":
            if name != partition_name:
                in_names.append(name)
        elif alloc.kind == "ExternalOutput":
            shape = tuple(alloc.tensor_shape)
            dtype = _mybir.dt.np(alloc.dtype)
            out_names.append(name)
            out_avals.append(jax.core.ShapedArray(shape, dtype))
            zero_outs.append(np.zeros(shape, dtype))
    n_params = len(in_names)
    all_in_names = in_names + out_names + ([partition_name] if partition_name else [])
    donate = tuple(range(n_params, n_params + len(out_names)))

    def _body(*args):
        operands = list(args)
        if partition_name is not None:
            operands.append(bass2jax.partition_id_tensor())
        return tuple(
            bass2jax._bass_exec_p.bind(
                *operands,
                out_avals=tuple(out_avals),
                in_names=tuple(all_in_names),
                out_names=tuple(out_names),
                lowering_input_output_aliases=(),
                sim_require_finite=True,
                sim_require_nnan=True,
                nc=nc,
            )
        )

    devices = jax.devices()[:N_CORES]
    mesh = Mesh(np.asarray(devices), ("core",))
    specs = (PartitionSpec("core"),) * (n_params + len(out_names))
    sharded = jax.jit(
        shard_map(
            _body,
            mesh=mesh,
            in_specs=specs,
            out_specs=(PartitionSpec("core"),) * len(out_names),
            check_rep=False,
        ),
        donate_argnums=donate,
        keep_unused=True,
    )

    def run(in_maps):
        concat_in = [
            np.concatenate([np.asarray(m[nm]) for m in in_maps], axis=0)
            for nm in in_names
        ]
        concat_zeros = [
            np.zeros((N_CORES * z.shape[0], *z.shape[1:]), z.dtype) for z in zero_outs
        ]
        out_arrs = sharded(*concat_in, *concat_zeros)
        return [
            {
                nm: np.asarray(out_arrs[i]).reshape(N_CORES, *out_avals[i].shape)[c]
                for i, nm in enumerate(out_names)
            }
            for c in range(N_CORES)
        ], (sharded, in_names, zero_outs, out_names, out_avals)

    _NC_CACHE["fn"] = run
    return run


def _make_in_maps(x, Wq, Wk, Wv):
    d = x.shape[-1]
    wqT = np.ascontiguousarray((Wq / np.sqrt(d)).T.astype(np.float32))
    wkT = np.ascontiguousarray(Wk.T.astype(np.float32))
    wvT = np.ascontiguousarray(Wv.T.astype(np.float32))
    in_maps = []
    for c in range(N_CORES):
        b, h = c // 2, c % 2
        xT = x[b].T  # [d, s]
        xT = np.ascontiguousarray(np.roll(xT, -h * 1024, axis=1), dtype=np.float32)
        in_maps.append({"xT": xT, "wqT": wqT, "wkT": wkT, "wvT": wvT})
    return in_maps


def _assemble(results, B, S, D):
    out = np.empty((B, S, D), dtype=np.float32)
    for c in range(N_CORES):
        b, h = c // 2, c % 2
        out[b, h * 1024 : (h + 1) * 1024, :] = results[c]["out"]
    return out


def kernel(x, Wq, Wk, Wv):
    x = np.asarray(x, dtype=np.float32)
    in_maps = _make_in_maps(x, Wq, Wk, Wv)
    results, _ = _get_sharded_fn()(in_maps)
    return _assemble(results, *x.shape)


def benchmark(x, Wq, Wk, Wv, iters=20):
    """Returns (out, per_call_seconds_list) timing the jitted sharded call."""
    import time
    import jax

    x = np.asarray(x, dtype=np.float32)
    in_maps = _make_in_maps(x, Wq, Wk, Wv)
    run = _get_sharded_fn()
    results, (sharded, in_names, zero_outs, out_names, out_avals) = run(in_maps)
    concat_in = [
        np.concatenate([np.asarray(m[nm]) for m in in_maps], axis=0)
        for nm in in_names
    ]
    concat_in = [jax.device_put(a) for a in concat_in]
    times = []
    for _ in range(iters):
        concat_zeros = [
            np.zeros((N_CORES * z.shape[0], *z.shape[1:]), z.dtype) for z in zero_outs
        ]
        t0 = time.perf_counter()
        out_arrs = sharded(*concat_in, *concat_zeros)
        jax.block_until_ready(out_arrs)
        times.append(time.perf_counter() - t0)
    return _assemble(results, *x.shape), times


# revision 18
# speedup vs baseline: 1021.5432x; 1021.5432x over previous
"""Classical single-head self-attention on 8 Trainium2 NeuronCores.

Problem (hardcoded): x [4, 2048, 1024] f32, Wq/Wk/Wv [1024, 1024] f32.
    q = x @ Wq.T ; k = x @ Wk.T ; v = x @ Wv.T
    out = softmax(q @ k.T / sqrt(1024)) @ v

Sharding: 8 cores = 4 batches x 2 query-halves. Core c handles batch
c//2 and queries [h*1024, (h+1)*1024) with h = c%2. Each core computes
k/v for its batch's full 2048-token sequence (duplicated across the
pair of cores sharing a batch); no collectives.

Host-side layout trick: each core receives xT = x[b].T (shape [1024 d,
2048 s]) with the sequence axis rolled so the core's own query half
always occupies columns [0, 1024). Attention is permutation-invariant
over keys, so rolling the key order changes nothing. Weights are passed
pre-transposed ([d, e]) and Wq is pre-scaled by 1/sqrt(d), so every
matmul has its contraction dim on SBUF partitions.

On-core pipeline (all matmuls fp32r except the attn@v stage in bf16):
  1. qT [e, sq=1024] = WqT.T @ xT[:, :1024]          (resident fp32r)
  2. kT [e, sk=2048] -> bounced to DRAM scratch      (SBUF too small)
  3. v  [sk, e]      = xT.T @ WvT                    (resident bf16)
  4. per sq-chunk of 512: scoresT [sk, sq'] = kT.T @ qT', exp via ACT
     (no max subtraction: scores are O(5) by construction),
     row-sums via ones-vector matmul, out = expT.T @ v scaled by 1/sum.
"""

import numpy as np
from contextlib import ExitStack

import concourse.bacc as bacc
import concourse.tile as tile
from concourse import mybir
from concourse.bass_utils import run_bass_kernel_spmd

F32 = mybir.dt.float32
F32R = mybir.dt.float32r
BF16 = mybir.dt.bfloat16
EXP = mybir.ActivationFunctionType.Exp

N_CORES = 8

GROUPS = [[0, 1], [2, 3], [4, 5], [6, 7]]


def emit_rep_v2(
    nc, tc, dram, qt_pool, v_pool, misc, mm_ps,
    P, DT, ET, KT, NCH, ECH, S, SQ, D, av_t,
    xT_d, wqT_d, wkT_d, wvT_d, out_d,
    cc_mock=False,
):
    def all_gather(in_ap, out_ap):
        if cc_mock:
            nc.sync.dma_start(out=out_ap[0], in_=in_ap)
            nc.sync.dma_start(out=out_ap[1], in_=in_ap)
        else:
            nc.gpsimd.collective_compute(
                "AllGather",
                mybir.AluOpType.bypass,
                replica_groups=GROUPS,
                ins=[in_ap],
                outs=[out_ap],
            )

    SH = S // 2           # tokens owned per core (= SQ)
    KTH = SH // P         # own sk tiles (8)
    NSC = SH // 512       # 512-token blocks per half (2)

    ktg_in = dram.tile([NSC, D, 512], F32R, tag="ktg_in", name="ktg_in")
    vg_in = dram.tile([SH, D], av_t, tag="vg_in", name="vg_in")
    ktg_out = dram.tile([NSC, 2, D, 512], F32R, tag="ktg_out", name="ktg_out")
    vg_out = dram.tile([2, SH, D], av_t, tag="vg_out", name="vg_out")
    sums_dram = dram.tile([NCH, 512], F32, tag="sums_d", name="sums_d")
    qt = qt_pool.tile([P, ET, SQ], F32R, tag="qt", name="qt")
    kt = v_pool.tile([P, ET, 2, SH], F32R, tag="kt", name="kt")
    ones = misc.tile([P, 1], av_t, tag="ones", name="ones")
    nc.vector.memset(ones, 1.0)

    # ---------------- projections ----------------
    with (
        tc.tile_pool(name="xt", bufs=1) as xt_pool,
        tc.tile_pool(name="w", bufs=2) as w_pool,
        tc.tile_pool(name="stage", bufs=3) as stage_pool,
    ):
        def load_xt_half(xt, half):
            cl, ch = half * (SH // 2), (half + 1) * (SH // 2)
            for dt_i in range(DT):
                nc.sync.dma_start(
                    out=xt[:, dt_i, cl:ch],
                    in_=xT_d[dt_i * P : (dt_i + 1) * P, cl:ch].bitcast(F32R),
                )

        def load_w(w_dram, nm):
            w_t = w_pool.tile([P, DT, D], F32R, tag="w", name=nm)
            for eh in range(2):
                el, er = eh * (D // 2), (eh + 1) * (D // 2)
                nc.sync.dma_start(
                    out=w_t[:, :, el:er],
                    in_=w_dram[:, el:er]
                    .rearrange("(dt p) e -> p dt e", p=P)
                    .bitcast(F32R),
                )
            return w_t

        xt = xt_pool.tile([P, DT, SH], F32R)
        load_xt_half(xt, 0)
        load_xt_half(xt, 1)
        wk = load_w(wkT_d, "wk")
        wq = load_w(wqT_d, "wq")

        # K stage, sk-block-major; exchange + reload pipelined per block
        for sc in range(NSC):
            blk = slice(sc * 512, (sc + 1) * 512)
            for et in range(ET):
                ps = mm_ps.tile([P, 512], F32, tag="mm", name="mm")
                for dt_i in range(DT):
                    nc.tensor.matmul(
                        ps,
                        wk[:, dt_i, et * P : (et + 1) * P],
                        xt[:, dt_i, blk],
                        start=(dt_i == 0),
                        stop=(dt_i == DT - 1),
                    )
                stg = stage_pool.tile([P, 512], F32R, tag="kstg", name="kstg")
                nc.vector.tensor_copy(stg, ps)
                nc.scalar.dma_start(
                    out=ktg_in[sc, et * P : (et + 1) * P, :], in_=stg
                )
            all_gather(ktg_in[sc], ktg_out[sc])
            for g in range(2):
                nc.sync.dma_start(
                    out=kt[:, :, g, blk],
                    in_=ktg_out[sc, g].rearrange("(et p) s -> p et s", p=P),
                )

        # Q, sq-chunk-major so scores chunk 0 unblocks at Q's midpoint
        for chk in range(NCH):
            for et in range(ET):
                ps = mm_ps.tile([P, 512], F32, tag="mm", name="mm")
                for dt_i in range(DT):
                    nc.tensor.matmul(
                        ps,
                        wq[:, dt_i, et * P : (et + 1) * P],
                        xt[:, dt_i, chk * 512 : (chk + 1) * 512],
                        start=(dt_i == 0),
                        stop=(dt_i == DT - 1),
                    )
                nc.vector.tensor_copy(qt[:, et, chk * 512 : (chk + 1) * 512], ps)

        # V (consumed late by AV; its gather hides behind scores)
        wv = load_w(wvT_d, "wv")
        for kt_i in range(KTH):
            stg = stage_pool.tile([P, D], av_t, tag="vstg", name="vstg")
            for ec in range(ECH):
                ps = mm_ps.tile([P, 512], F32, tag="mm", name="mm")
                for dt_i in range(DT):
                    nc.tensor.matmul(
                        ps,
                        xt[:, dt_i, kt_i * P : (kt_i + 1) * P],
                        wv[:, dt_i, ec * 512 : (ec + 1) * 512],
                        start=(dt_i == 0),
                        stop=(dt_i == DT - 1),
                    )
                nc.scalar.copy(stg[:, ec * 512 : (ec + 1) * 512], ps)
            nc.scalar.dma_start(out=vg_in[kt_i * P : (kt_i + 1) * P, :], in_=stg)

        all_gather(vg_in[:], vg_out[:])

    # ---------------- attention ----------------
    with (
        tc.tile_pool(name="vres", bufs=1) as vres_pool,
        tc.tile_pool(name="expt", bufs=2) as expt_pool,
        tc.tile_pool(name="osb", bufs=3) as osb_pool,
        tc.tile_pool(name="sums", bufs=2) as sums_pool,
        tc.tile_pool(name="scps", bufs=3, space="PSUM") as sc_ps,
        tc.tile_pool(name="smps", bufs=1, space="PSUM") as sm_ps,
    ):
        v = vres_pool.tile([P, KT, D], av_t, tag="v", name="v")
        nc.sync.dma_start(
            out=v, in_=vg_out[:].rearrange("g (t p) d -> p (g t) d", p=P)
        )

        for chk in range(NCH):
            sq_lo = chk * 512
            expt = expt_pool.tile([P, KT, 512], av_t, tag="expt", name="expt")
            sums_ps = sm_ps.tile([1, 512], F32, tag="sums", name="sums_ps")
            for kt_i in range(KT):
                g, off = kt_i // KTH, (kt_i % KTH) * P
                ps = sc_ps.tile([P, 512], F32, tag="sc", name="sc")
                for et in range(ET):
                    nc.tensor.matmul(
                        ps,
                        kt[:, et, g, off : off + P],
                        qt[:, et, sq_lo : sq_lo + 512],
                        start=(et == 0),
                        stop=(et == ET - 1),
                    )
                nc.scalar.activation(expt[:, kt_i, :], ps, EXP)
                nc.tensor.matmul(
                    sums_ps,
                    ones,
                    expt[:, kt_i, :],
                    start=(kt_i == 0),
                    stop=(kt_i == KT - 1),
                )

            recipT = sums_pool.tile([P, 4], F32, tag="recipT", name="recipT")
            sums_sb = sums_pool.tile([1, 512], F32, tag="sums_sb", name="sums_sb")
            nc.scalar.copy(sums_sb, sums_ps)
            nc.sync.dma_start(out=sums_dram[chk], in_=sums_sb[:])
            nc.sync.dma_start(
                out=recipT, in_=sums_dram[chk].rearrange("(j p) -> p j", p=P)
            )
            nc.vector.reciprocal(recipT, recipT)

            for st in range(4):
                osb = osb_pool.tile([P, D], F32, tag="osb", name="osb")
                for ec in range(ECH):
                    ps = mm_ps.tile([P, 512], F32, tag="mm", name="mm")
                    for kt_i in range(KT):
                        nc.tensor.matmul(
                            ps,
                            expt[:, kt_i, st * P : (st + 1) * P],
                            v[:, kt_i, ec * 512 : (ec + 1) * 512],
                            start=(kt_i == 0),
                            stop=(kt_i == KT - 1),
                        )
                    nc.vector.tensor_scalar_mul(
                        osb[:, ec * 512 : (ec + 1) * 512], ps, recipT[:, st : st + 1]
                    )
                row = sq_lo + st * P
                nc.scalar.dma_start(out=out_d[row : row + P, :], in_=osb)




def build_nc(D=1024, S=2048, SQ=1024, av_bf16=True, debug_skip=(), reps=1, kv_exchange=False, cc_mock=False):
    """Emit the per-core kernel. D = embed dim, S = keys, SQ = queries.

    reps>1 re-emits the whole body; bufs=1 pool reuse makes the reps run
    near-serially, which lets wall-clock deltas measure per-rep HW time.
    """
    P = 128
    DT = D // P          # d tiles (contraction for projections)
    ET = D // P          # e tiles
    KT = S // P          # sk tiles
    NCH = SQ // 512      # sq chunks of 512
    ECH = D // 512       # e chunks of 512

    nc = bacc.Bacc("TRN2", target_bir_lowering=False)

    xT_d = nc.dram_tensor(
        "xT", [D, S // 2 if kv_exchange else S], F32, kind="ExternalInput"
    )
    wqT_d = nc.dram_tensor("wqT", [D, D], F32, kind="ExternalInput")
    wkT_d = nc.dram_tensor("wkT", [D, D], F32, kind="ExternalInput")
    wvT_d = nc.dram_tensor("wvT", [D, D], F32, kind="ExternalInput")
    out_d = nc.dram_tensor("out", [SQ, D], F32, kind="ExternalOutput")

    av_t = BF16 if av_bf16 else F32

    with ExitStack() as ctx:
        tc = ctx.enter_context(tile.TileContext(nc))
        dram = ctx.enter_context(tc.tile_pool(name="dram", bufs=1, space="DRAM"))
        qt_pool = ctx.enter_context(tc.tile_pool(name="qt", bufs=1))
        v_pool = ctx.enter_context(tc.tile_pool(name="v", bufs=1))
        misc = ctx.enter_context(tc.tile_pool(name="misc", bufs=1))
        mm_ps = ctx.enter_context(tc.tile_pool(name="mmps", bufs=4, space="PSUM"))

        for _rep in range(reps):
            if kv_exchange:
                emit_rep_v2(
                    nc, tc, dram, qt_pool, v_pool, misc, mm_ps,
                    P, DT, ET, KT, NCH, ECH, S, SQ, D, av_t,
                    xT_d, wqT_d, wkT_d, wvT_d, out_d,
                    cc_mock=cc_mock,
                )
            else:
                _emit_rep(
                    nc, tc, dram, qt_pool, v_pool, misc, mm_ps,
                    P, DT, ET, KT, NCH, ECH, S, SQ, D, av_t, debug_skip,
                    xT_d, wqT_d, wkT_d, wvT_d, out_d,
                )

    nc.compile()
    return nc


def _emit_rep(
    nc, tc, dram, qt_pool, v_pool, misc, mm_ps,
    P, DT, ET, KT, NCH, ECH, S, SQ, D, av_t, debug_skip,
    xT_d, wqT_d, wkT_d, wvT_d, out_d,
):
    kT_dram = dram.tile([D, S], F32R, tag="ktd", name="ktd")
    sums_dram = dram.tile([NCH, 512], F32, tag="sums_d", name="sums_d")
    qt = qt_pool.tile([P, ET, SQ], F32R, tag="qt", name="qt")
    v = v_pool.tile([P, KT, D], av_t, tag="v", name="v")
    ones = misc.tile([P, 1], av_t, tag="ones", name="ones")
    nc.vector.memset(ones, 1.0)

    # ---------------- projections ----------------
    with (
        tc.tile_pool(name="xt", bufs=1) as xt_pool,
        tc.tile_pool(name="w", bufs=12) as w_pool,
        tc.tile_pool(name="ktsb", bufs=3) as ktsb_pool,
    ):
        xt = xt_pool.tile([P, DT, S], F32R)
        nc.sync.dma_start(
            out=xt, in_=xT_d[:].rearrange("(dt p) s -> p dt s", p=P).bitcast(F32R)
        )

        def load_w(w_dram):
            tiles = []
            for dt_i in range(DT):
                w_t = w_pool.tile([P, D], F32R, tag="w", name="w")
                nc.sync.dma_start(
                    out=w_t, in_=w_dram[dt_i * P : (dt_i + 1) * P, :].bitcast(F32R)
                )
                tiles.append(w_t)
            return tiles

        wq = load_w(wqT_d)
        wk = load_w(wkT_d)
        wv = load_w(wvT_d)

        # qT[e, sq]: lhsT = wqT[d, e-tile], rhs = xT[d, sq-chunk]
        for et in range(ET):
            for chk in range(NCH):
                ps = mm_ps.tile([P, 512], F32, tag="mm", name="mm")
                for dt_i in range(DT):
                    nc.tensor.matmul(
                        ps,
                        wq[dt_i][:, et * P : (et + 1) * P],
                        xt[:, dt_i, chk * 512 : (chk + 1) * 512],
                        start=(dt_i == 0),
                        stop=(dt_i == DT - 1),
                    )
                nc.vector.tensor_copy(qt[:, et, chk * 512 : (chk + 1) * 512], ps)

        # kT[e, sk] -> DRAM scratch
        for et in range(ET):
            for sc in range(S // 512):
                ps = mm_ps.tile([P, 512], F32, tag="mm", name="mm")
                for dt_i in range(DT):
                    nc.tensor.matmul(
                        ps,
                        wk[dt_i][:, et * P : (et + 1) * P],
                        xt[:, dt_i, sc * 512 : (sc + 1) * 512],
                        start=(dt_i == 0),
                        stop=(dt_i == DT - 1),
                    )
                kt_sb = ktsb_pool.tile([P, 512], F32R, tag="ktsb", name="ktsb")
                nc.vector.tensor_copy(kt_sb, ps)
                nc.sync.dma_start(
                    out=kT_dram[et * P : (et + 1) * P, sc * 512 : (sc + 1) * 512],
                    in_=kt_sb,
                )

        # v[sk, e]: lhsT = xT[d, sk-tile], rhs = wvT[d, e-chunk]
        for kt_i in range(KT):
            for ec in range(ECH):
                ps = mm_ps.tile([P, 512], F32, tag="mm", name="mm")
                for dt_i in range(DT):
                    nc.tensor.matmul(
                        ps,
                        xt[:, dt_i, kt_i * P : (kt_i + 1) * P],
                        wv[dt_i][:, ec * 512 : (ec + 1) * 512],
                        start=(dt_i == 0),
                        stop=(dt_i == DT - 1),
                    )
                nc.scalar.copy(v[:, kt_i, ec * 512 : (ec + 1) * 512], ps)

    # ---------------- attention ----------------
    if "attn" in debug_skip:
        dummy = qt_pool.tile([P, 512], F32, tag="dummy", name="dummy")
        nc.vector.tensor_copy(dummy, qt[:, 0, 0:512])
        nc.sync.dma_start(out=out_d[0:P, 0:512], in_=dummy)
        return
    with (
        tc.tile_pool(name="ktin", bufs=4) as ktin_pool,
        tc.tile_pool(name="expt", bufs=2) as expt_pool,
        tc.tile_pool(name="osb", bufs=3) as osb_pool,
        tc.tile_pool(name="sums", bufs=2) as sums_pool,
        tc.tile_pool(name="scps", bufs=3, space="PSUM") as sc_ps,
        tc.tile_pool(name="smps", bufs=1, space="PSUM") as sm_ps,
    ):
        kT_r = kT_dram[:].rearrange("(et p) s -> p et s", p=P)
        for chk in range(NCH):
            sq_lo = chk * 512
            expt = expt_pool.tile([P, KT, 512], av_t, tag="expt", name="expt")
            if "sums" not in debug_skip:
                sums_ps = sm_ps.tile([1, 512], F32, tag="sums", name="sums_ps")
            for kt_i in range(KT):
                ktt = ktin_pool.tile([P, ET, P], F32R, tag="ktin", name="ktin")
                nc.sync.dma_start(out=ktt, in_=kT_r[:, :, kt_i * P : (kt_i + 1) * P])
                ps = sc_ps.tile([P, 512], F32, tag="sc", name="sc")
                for et in range(ET):
                    nc.tensor.matmul(
                        ps,
                        ktt[:, et, :],
                        qt[:, et, sq_lo : sq_lo + 512],
                        start=(et == 0),
                        stop=(et == ET - 1),
                    )
                if "exp" in debug_skip:
                    nc.scalar.copy(expt[:, kt_i, :], ps)
                else:
                    nc.scalar.activation(expt[:, kt_i, :], ps, EXP)
                if "sums" not in debug_skip:
                    nc.tensor.matmul(
                        sums_ps,
                        ones,
                        expt[:, kt_i, :],
                        start=(kt_i == 0),
                        stop=(kt_i == KT - 1),
                    )

            # sums [1, 512] -> recipT [128, 4] via DRAM bounce
            recipT = sums_pool.tile([P, 4], F32, tag="recipT", name="recipT")
            if "sums" in debug_skip:
                nc.vector.memset(recipT, 1.0)
            else:
                sums_sb = sums_pool.tile([1, 512], F32, tag="sums_sb", name="sums_sb")
                nc.scalar.copy(sums_sb, sums_ps)
                nc.sync.dma_start(out=sums_dram[chk], in_=sums_sb[:])
                nc.sync.dma_start(
                    out=recipT,
                    in_=sums_dram[chk].rearrange("(j p) -> p j", p=P),
                )
                nc.vector.reciprocal(recipT, recipT)

            # out[sq, e] = expT.T @ v, scaled by 1/rowsum
            for st in range(4):
                for ec in range(ECH):
                    ps = mm_ps.tile([P, 512], F32, tag="mm", name="mm")
                    for kt_i in range(KT):
                        nc.tensor.matmul(
                            ps,
                            expt[:, kt_i, st * P : (st + 1) * P],
                            v[:, kt_i, ec * 512 : (ec + 1) * 512],
                            start=(kt_i == 0),
                            stop=(kt_i == KT - 1),
                        )
                    osb = osb_pool.tile([P, 512], F32, tag="osb", name="osb")
                    nc.vector.tensor_scalar_mul(osb, ps, recipT[:, st : st + 1])
                    row = sq_lo + st * P
                    nc.sync.dma_start(
                        out=out_d[row : row + P, ec * 512 : (ec + 1) * 512],
                        in_=osb,
                    )


_NC_CACHE = {}


KV_EXCHANGE = True


def _get_nc(reps=1):
    key = ("nc", reps, KV_EXCHANGE)
    if key not in _NC_CACHE:
        _NC_CACHE[key] = build_nc(reps=reps, kv_exchange=KV_EXCHANGE)
    return _NC_CACHE[key]


def _get_sharded_fn(reps=1):
    """jit-once 8-core executor mirroring bass2jax.run_bass_via_pjrt."""
    key = ("fn", reps, KV_EXCHANGE)
    if key in _NC_CACHE:
        return _NC_CACHE[key]
    import jax
    from jax.experimental.shard_map import shard_map
    from jax.sharding import Mesh, PartitionSpec
    from concourse import mybir as _mybir
    from concourse import bass2jax

    nc = _get_nc(reps)
    bass2jax.install_neuronx_cc_hook()
    partition_name = nc.partition_id_tensor.name if nc.partition_id_tensor else None
    in_names, out_names, out_avals, zero_outs = [], [], [], []
    for alloc in nc.m.functions[0].allocations:
        if not isinstance(alloc, _mybir.MemoryLocationSet):
            continue
        name = alloc.memorylocations[0].name
        if alloc.kind == "ExternalInput":
            if name != partition_name:
                in_names.append(name)
        elif alloc.kind == "ExternalOutput":
            shape = tuple(alloc.tensor_shape)
            dtype = _mybir.dt.np(alloc.dtype)
            out_names.append(name)
            out_avals.append(jax.core.ShapedArray(shape, dtype))
            zero_outs.append(np.zeros(shape, dtype))
    n_params = len(in_names)
    all_in_names = in_names + out_names + ([partition_name] if partition_name else [])
    donate = tuple(range(n_params, n_params + len(out_names)))

    def _body(*args):
        operands = list(args)
        if partition_name is not None:
            operands.append(bass2jax.partition_id_tensor())
        return tuple(
            bass2jax._bass_exec_p.bind(
                *operands,
                out_avals=tuple(out_avals),
                in_names=tuple(all_in_names),
                out_names=tuple(out_names),
                lowering_input_output_aliases=(),
                sim_require_finite=True,
                sim_require_nnan=True,
                nc=nc,
            )
        )

    devices = jax.devices()[:N_CORES]
    mesh = Mesh(np.asarray(devices), ("core",))
    specs = (PartitionSpec("core"),) * (n_params + len(out_names))
    sharded = jax.jit(
        shard_map(
            _body,
            mesh=mesh,
            in_specs=specs,
            out_specs=(PartitionSpec("core"),) * len(out_names),
            check_rep=False,
        ),
        donate_argnums=donate,
        keep_unused=True,
    )

    class Runner:
        pass

    r = Runner()
    r.sharded = sharded
    r.in_names = in_names
    r.out_names = out_names
    r.out_avals = out_avals
    r.zero_outs = zero_outs
    r.mesh = mesh

    def run(in_maps):
        concat_in = [
            np.concatenate([np.asarray(m[nm]) for m in in_maps], axis=0)
            for nm in in_names
        ]
        concat_zeros = [
            np.zeros((N_CORES * z.shape[0], *z.shape[1:]), z.dtype) for z in zero_outs
        ]
        out_arrs = sharded(*concat_in, *concat_zeros)
        return [
            {
                nm: np.asarray(out_arrs[i]).reshape(N_CORES, *out_avals[i].shape)[c]
                for i, nm in enumerate(out_names)
            }
            for c in range(N_CORES)
        ]

    r.run = run
    _NC_CACHE[key] = r
    return r


def _make_in_maps(x, Wq, Wk, Wv):
    d = x.shape[-1]
    wqT = np.ascontiguousarray((Wq / np.sqrt(d)).T.astype(np.float32))
    wkT = np.ascontiguousarray(Wk.T.astype(np.float32))
    wvT = np.ascontiguousarray(Wv.T.astype(np.float32))
    in_maps = []
    for c in range(N_CORES):
        b, h = c // 2, c % 2
        xT = x[b].T  # [d, s]
        if KV_EXCHANGE:
            # own-half columns only; partner k/v arrive via AllGather
            xT = np.ascontiguousarray(xT[:, h * 1024 : (h + 1) * 1024], np.float32)
        else:
            xT = np.ascontiguousarray(np.roll(xT, -h * 1024, axis=1), np.float32)
        in_maps.append({"xT": xT, "wqT": wqT, "wkT": wkT, "wvT": wvT})
    return in_maps


def _assemble(results, B, S, D):
    out = np.empty((B, S, D), dtype=np.float32)
    for c in range(N_CORES):
        b, h = c // 2, c % 2
        out[b, h * 1024 : (h + 1) * 1024, :] = results[c]["out"]
    return out


def kernel(x, Wq, Wk, Wv):
    x = np.asarray(x, dtype=np.float32)
    in_maps = _make_in_maps(x, Wq, Wk, Wv)
    results = _get_sharded_fn().run(in_maps)
    return _assemble(results, *x.shape)


def bench_reps(x, Wq, Wk, Wv, reps, iters=10):
    """Time the sharded call with device-resident inputs. Returns seconds list."""
    import time
    import jax
    from jax.sharding import NamedSharding, PartitionSpec

    x = np.asarray(x, dtype=np.float32)
    in_maps = _make_in_maps(x, Wq, Wk, Wv)
    r = _get_sharded_fn(reps)
    concat_in = [
        np.concatenate([np.asarray(m[nm]) for m in in_maps], axis=0)
        for nm in r.in_names
    ]
    shard = NamedSharding(r.mesh, PartitionSpec("core"))
    dev_in = [jax.device_put(a, shard) for a in concat_in]
    times = []
    out = None
    for _ in range(iters):
        concat_zeros = [
            jax.device_put(
                np.zeros((N_CORES * z.shape[0], *z.shape[1:]), z.dtype), shard
            )
            for z in r.zero_outs
        ]
        jax.block_until_ready(concat_zeros)
        t0 = time.perf_counter()
        out_arrs = r.sharded(*dev_in, *concat_zeros)
        jax.block_until_ready(out_arrs)
        times.append(time.perf_counter() - t0)
        out = out_arrs
    results = [
        {
            nm: np.asarray(out[i]).reshape(N_CORES, *r.out_avals[i].shape)[c]
            for i, nm in enumerate(r.out_names)
        }
        for c in range(N_CORES)
    ]
    return _assemble(results, *x.shape), times


# revision 19
# speedup vs baseline: 1609.5493x; 1.5756x over previous
"""Classical single-head self-attention on 8 Trainium2 NeuronCores.

Problem (hardcoded): x [4, 2048, 1024] f32, Wq/Wk/Wv [1024, 1024] f32.
    q = x @ Wq.T ; k = x @ Wk.T ; v = x @ Wv.T
    out = softmax(q @ k.T / sqrt(1024)) @ v

Sharding: 8 cores = 4 batches x 2 query-halves. Core c handles batch
c//2 and queries [h*1024, (h+1)*1024) with h = c%2. Each core computes
k/v for its batch's full 2048-token sequence (duplicated across the
pair of cores sharing a batch); no collectives.

Host-side layout trick: each core receives xT = x[b].T (shape [1024 d,
2048 s]) with the sequence axis rolled so the core's own query half
always occupies columns [0, 1024). Attention is permutation-invariant
over keys, so rolling the key order changes nothing. Weights are passed
pre-transposed ([d, e]) and Wq is pre-scaled by 1/sqrt(d), so every
matmul has its contraction dim on SBUF partitions.

On-core pipeline (all matmuls fp32r except the attn@v stage in bf16):
  1. qT [e, sq=1024] = WqT.T @ xT[:, :1024]          (resident fp32r)
  2. kT [e, sk=2048] -> bounced to DRAM scratch      (SBUF too small)
  3. v  [sk, e]      = xT.T @ WvT                    (resident bf16)
  4. per sq-chunk of 512: scoresT [sk, sq'] = kT.T @ qT', exp via ACT
     (no max subtraction: scores are O(5) by construction),
     row-sums via ones-vector matmul, out = expT.T @ v scaled by 1/sum.
"""

import numpy as np
from contextlib import ExitStack

import concourse.bacc as bacc
import concourse.tile as tile
from concourse import mybir
from concourse.bass_utils import run_bass_kernel_spmd

F32 = mybir.dt.float32
F32R = mybir.dt.float32r
BF16 = mybir.dt.bfloat16
EXP = mybir.ActivationFunctionType.Exp

N_CORES = 8

GROUPS = [[0, 1], [2, 3], [4, 5], [6, 7]]


def emit_rep_v2(
    nc, tc, dram, qt_pool, v_pool, misc, mm_ps,
    P, DT, ET, KT, NCH, ECH, S, SQ, D, av_t,
    xT_d, wqT_d, wkT_d, wvT_d, out_d,
    cc_mock=False,
):
    def all_gather(in_ap, out_ap):
        if cc_mock:
            nc.sync.dma_start(out=out_ap[0], in_=in_ap)
            nc.sync.dma_start(out=out_ap[1], in_=in_ap)
        else:
            nc.gpsimd.collective_compute(
                "AllGather",
                mybir.AluOpType.bypass,
                replica_groups=GROUPS,
                ins=[in_ap],
                outs=[out_ap],
            )

    SH = S // 2           # tokens owned per core (= SQ)
    KTH = SH // P         # own sk tiles (8)
    NSC = SH // 512       # 512-token blocks per half (2)

    ktg_in = dram.tile([NSC, D, 512], F32R, tag="ktg_in", name="ktg_in")
    vg_in = dram.tile([SH, D], av_t, tag="vg_in", name="vg_in")
    ktg_out = dram.tile([NSC, 2, D, 512], F32R, tag="ktg_out", name="ktg_out")
    vg_out = dram.tile([2, SH, D], av_t, tag="vg_out", name="vg_out")
    sums_dram = dram.tile([NCH, 512], F32, tag="sums_d", name="sums_d")
    qt = qt_pool.tile([P, ET, SQ], F32R, tag="qt", name="qt")
    kt = v_pool.tile([P, ET, 2, SH], F32R, tag="kt", name="kt")
    ones = misc.tile([P, 1], av_t, tag="ones", name="ones")
    nc.vector.memset(ones, 1.0)

    # ---------------- projections ----------------
    with (
        tc.tile_pool(name="xt", bufs=1) as xt_pool,
        tc.tile_pool(name="w", bufs=2) as w_pool,
        tc.tile_pool(name="stage", bufs=3) as stage_pool,
    ):
        def load_xt_half(xt, half):
            cl, ch = half * (SH // 2), (half + 1) * (SH // 2)
            for dt_i in range(DT):
                nc.sync.dma_start(
                    out=xt[:, dt_i, cl:ch],
                    in_=xT_d[dt_i * P : (dt_i + 1) * P, cl:ch].bitcast(F32R),
                )

        def load_w(w_dram, nm):
            w_t = w_pool.tile([P, DT, D], F32R, tag="w", name=nm)
            for eh in range(2):
                el, er = eh * (D // 2), (eh + 1) * (D // 2)
                nc.sync.dma_start(
                    out=w_t[:, :, el:er],
                    in_=w_dram[:, el:er]
                    .rearrange("(dt p) e -> p dt e", p=P)
                    .bitcast(F32R),
                )
            return w_t

        xt = xt_pool.tile([P, DT, SH], F32R)
        wk = load_w(wkT_d, "wk")
        load_xt_half(xt, 0)
        load_xt_half(xt, 1)
        wq = load_w(wqT_d, "wq")

        # K stage, sk-block-major; exchange + reload pipelined per block
        for sc in range(NSC):
            blk = slice(sc * 512, (sc + 1) * 512)
            for et in range(ET):
                ps = mm_ps.tile([P, 512], F32, tag="mm", name="mm")
                for dt_i in range(DT):
                    nc.tensor.matmul(
                        ps,
                        wk[:, dt_i, et * P : (et + 1) * P],
                        xt[:, dt_i, blk],
                        start=(dt_i == 0),
                        stop=(dt_i == DT - 1),
                    )
                stg = stage_pool.tile([P, 512], F32R, tag="kstg", name="kstg")
                nc.vector.tensor_copy(stg, ps)
                nc.scalar.dma_start(
                    out=ktg_in[sc, et * P : (et + 1) * P, :], in_=stg
                )
            all_gather(ktg_in[sc], ktg_out[sc])
            for g in range(2):
                nc.scalar.dma_start(
                    out=kt[:, :, g, blk],
                    in_=ktg_out[sc, g].rearrange("(et p) s -> p et s", p=P),
                )

        # Q, sq-chunk-major so scores chunk 0 unblocks at Q's midpoint
        for chk in range(NCH):
            for et in range(ET):
                ps = mm_ps.tile([P, 512], F32, tag="mm", name="mm")
                for dt_i in range(DT):
                    nc.tensor.matmul(
                        ps,
                        wq[:, dt_i, et * P : (et + 1) * P],
                        xt[:, dt_i, chk * 512 : (chk + 1) * 512],
                        start=(dt_i == 0),
                        stop=(dt_i == DT - 1),
                    )
                nc.vector.tensor_copy(qt[:, et, chk * 512 : (chk + 1) * 512], ps)

        # V (consumed late by AV; its gather hides behind scores)
        wv = load_w(wvT_d, "wv")
        for kt_i in range(KTH):
            stg = stage_pool.tile([P, D], av_t, tag="vstg", name="vstg")
            for ec in range(ECH):
                ps = mm_ps.tile([P, 512], F32, tag="mm", name="mm")
                for dt_i in range(DT):
                    nc.tensor.matmul(
                        ps,
                        xt[:, dt_i, kt_i * P : (kt_i + 1) * P],
                        wv[:, dt_i, ec * 512 : (ec + 1) * 512],
                        start=(dt_i == 0),
                        stop=(dt_i == DT - 1),
                    )
                nc.scalar.copy(stg[:, ec * 512 : (ec + 1) * 512], ps)
            nc.scalar.dma_start(out=vg_in[kt_i * P : (kt_i + 1) * P, :], in_=stg)

        all_gather(vg_in[:], vg_out[:])

    # ---------------- attention ----------------
    with (
        tc.tile_pool(name="vres", bufs=1) as vres_pool,
        tc.tile_pool(name="expt", bufs=2) as expt_pool,
        tc.tile_pool(name="osb", bufs=3) as osb_pool,
        tc.tile_pool(name="sums", bufs=2) as sums_pool,
        tc.tile_pool(name="scps", bufs=3, space="PSUM") as sc_ps,
        tc.tile_pool(name="smps", bufs=1, space="PSUM") as sm_ps,
    ):
        v = vres_pool.tile([P, KT, D], av_t, tag="v", name="v")
        nc.sync.dma_start(
            out=v, in_=vg_out[:].rearrange("g (t p) d -> p (g t) d", p=P)
        )

        for chk in range(NCH):
            sq_lo = chk * 512
            expt = expt_pool.tile([P, KT, 512], av_t, tag="expt", name="expt")
            sums_ps = sm_ps.tile([1, 512], F32, tag="sums", name="sums_ps")
            for kt_i in range(KT):
                g, off = kt_i // KTH, (kt_i % KTH) * P
                ps = sc_ps.tile([P, 512], F32, tag="sc", name="sc")
                for et in range(ET):
                    nc.tensor.matmul(
                        ps,
                        kt[:, et, g, off : off + P],
                        qt[:, et, sq_lo : sq_lo + 512],
                        start=(et == 0),
                        stop=(et == ET - 1),
                    )
                nc.scalar.activation(expt[:, kt_i, :], ps, EXP)
                nc.tensor.matmul(
                    sums_ps,
                    ones,
                    expt[:, kt_i, :],
                    start=(kt_i == 0),
                    stop=(kt_i == KT - 1),
                )

            recipT = sums_pool.tile([P, 4], F32, tag="recipT", name="recipT")
            sums_sb = sums_pool.tile([1, 512], F32, tag="sums_sb", name="sums_sb")
            nc.scalar.copy(sums_sb, sums_ps)
            nc.sync.dma_start(out=sums_dram[chk], in_=sums_sb[:])
            nc.sync.dma_start(
                out=recipT, in_=sums_dram[chk].rearrange("(j p) -> p j", p=P)
            )
            nc.vector.reciprocal(recipT, recipT)

            for st in range(4):
                osb = osb_pool.tile([P, D], F32, tag="osb", name="osb")
                for ec in range(ECH):
                    ps = mm_ps.tile([P, 512], F32, tag="mm", name="mm")
                    for kt_i in range(KT):
                        nc.tensor.matmul(
                            ps,
                            expt[:, kt_i, st * P : (st + 1) * P],
                            v[:, kt_i, ec * 512 : (ec + 1) * 512],
                            start=(kt_i == 0),
                            stop=(kt_i == KT - 1),
                        )
                    nc.vector.tensor_scalar_mul(
                        osb[:, ec * 512 : (ec + 1) * 512], ps, recipT[:, st : st + 1]
                    )
                row = sq_lo + st * P
                nc.scalar.dma_start(out=out_d[row : row + P, :], in_=osb)




def build_nc(D=1024, S=2048, SQ=1024, av_bf16=True, debug_skip=(), reps=1, kv_exchange=False, cc_mock=False):
    """Emit the per-core kernel. D = embed dim, S = keys, SQ = queries.

    reps>1 re-emits the whole body; bufs=1 pool reuse makes the reps run
    near-serially, which lets wall-clock deltas measure per-rep HW time.
    """
    P = 128
    DT = D // P          # d tiles (contraction for projections)
    ET = D // P          # e tiles
    KT = S // P          # sk tiles
    NCH = SQ // 512      # sq chunks of 512
    ECH = D // 512       # e chunks of 512

    nc = bacc.Bacc("TRN2", target_bir_lowering=False)

    xT_d = nc.dram_tensor(
        "xT", [D, S // 2 if kv_exchange else S], F32, kind="ExternalInput"
    )
    wqT_d = nc.dram_tensor("wqT", [D, D], F32, kind="ExternalInput")
    wkT_d = nc.dram_tensor("wkT", [D, D], F32, kind="ExternalInput")
    wvT_d = nc.dram_tensor("wvT", [D, D], F32, kind="ExternalInput")
    out_d = nc.dram_tensor("out", [SQ, D], F32, kind="ExternalOutput")

    av_t = BF16 if av_bf16 else F32

    with ExitStack() as ctx:
        tc = ctx.enter_context(tile.TileContext(nc))
        dram = ctx.enter_context(tc.tile_pool(name="dram", bufs=1, space="DRAM"))
        qt_pool = ctx.enter_context(tc.tile_pool(name="qt", bufs=1))
        v_pool = ctx.enter_context(tc.tile_pool(name="v", bufs=1))
        misc = ctx.enter_context(tc.tile_pool(name="misc", bufs=1))
        mm_ps = ctx.enter_context(tc.tile_pool(name="mmps", bufs=4, space="PSUM"))

        for _rep in range(reps):
            if kv_exchange:
                emit_rep_v2(
                    nc, tc, dram, qt_pool, v_pool, misc, mm_ps,
                    P, DT, ET, KT, NCH, ECH, S, SQ, D, av_t,
                    xT_d, wqT_d, wkT_d, wvT_d, out_d,
                    cc_mock=cc_mock,
                )
            else:
                _emit_rep(
                    nc, tc, dram, qt_pool, v_pool, misc, mm_ps,
                    P, DT, ET, KT, NCH, ECH, S, SQ, D, av_t, debug_skip,
                    xT_d, wqT_d, wkT_d, wvT_d, out_d,
                )

    nc.compile()
    return nc


def _emit_rep(
    nc, tc, dram, qt_pool, v_pool, misc, mm_ps,
    P, DT, ET, KT, NCH, ECH, S, SQ, D, av_t, debug_skip,
    xT_d, wqT_d, wkT_d, wvT_d, out_d,
):
    kT_dram = dram.tile([D, S], F32R, tag="ktd", name="ktd")
    sums_dram = dram.tile([NCH, 512], F32, tag="sums_d", name="sums_d")
    qt = qt_pool.tile([P, ET, SQ], F32R, tag="qt", name="qt")
    v = v_pool.tile([P, KT, D], av_t, tag="v", name="v")
    ones = misc.tile([P, 1], av_t, tag="ones", name="ones")
    nc.vector.memset(ones, 1.0)

    # ---------------- projections ----------------
    with (
        tc.tile_pool(name="xt", bufs=1) as xt_pool,
        tc.tile_pool(name="w", bufs=12) as w_pool,
        tc.tile_pool(name="ktsb", bufs=3) as ktsb_pool,
    ):
        xt = xt_pool.tile([P, DT, S], F32R)
        nc.sync.dma_start(
            out=xt, in_=xT_d[:].rearrange("(dt p) s -> p dt s", p=P).bitcast(F32R)
        )

        def load_w(w_dram):
            tiles = []
            for dt_i in range(DT):
                w_t = w_pool.tile([P, D], F32R, tag="w", name="w")
                nc.sync.dma_start(
                    out=w_t, in_=w_dram[dt_i * P : (dt_i + 1) * P, :].bitcast(F32R)
                )
                tiles.append(w_t)
            return tiles

        wq = load_w(wqT_d)
        wk = load_w(wkT_d)
        wv = load_w(wvT_d)

        # qT[e, sq]: lhsT = wqT[d, e-tile], rhs = xT[d, sq-chunk]
        for et in range(ET):
            for chk in range(NCH):
                ps = mm_ps.tile([P, 512], F32, tag="mm", name="mm")
                for dt_i in range(DT):
                    nc.tensor.matmul(
                        ps,
                        wq[dt_i][:, et * P : (et + 1) * P],
                        xt[:, dt_i, chk * 512 : (chk + 1) * 512],
                        start=(dt_i == 0),
                        stop=(dt_i == DT - 1),
                    )
                nc.vector.tensor_copy(qt[:, et, chk * 512 : (chk + 1) * 512], ps)

        # kT[e, sk] -> DRAM scratch
        for et in range(ET):
            for sc in range(S // 512):
                ps = mm_ps.tile([P, 512], F32, tag="mm", name="mm")
                for dt_i in range(DT):
                    nc.tensor.matmul(
                        ps,
                        wk[dt_i][:, et * P : (et + 1) * P],
                        xt[:, dt_i, sc * 512 : (sc + 1) * 512],
                        start=(dt_i == 0),
                        stop=(dt_i == DT - 1),
                    )
                kt_sb = ktsb_pool.tile([P, 512], F32R, tag="ktsb", name="ktsb")
                nc.vector.tensor_copy(kt_sb, ps)
                nc.sync.dma_start(
                    out=kT_dram[et * P : (et + 1) * P, sc * 512 : (sc + 1) * 512],
                    in_=kt_sb,
                )

        # v[sk, e]: lhsT = xT[d, sk-tile], rhs = wvT[d, e-chunk]
        for kt_i in range(KT):
            for ec in range(ECH):
                ps = mm_ps.tile([P, 512], F32, tag="mm", name="mm")
                for dt_i in range(DT):
                    nc.tensor.matmul(
                        ps,
                        xt[:, dt_i, kt_i * P : (kt_i + 1) * P],
                        wv[dt_i][:, ec * 512 : (ec + 1) * 512],
                        start=(dt_i == 0),
                        stop=(dt_i == DT - 1),
                    )
                nc.scalar.copy(v[:, kt_i, ec * 512 : (ec + 1) * 512], ps)

    # ---------------- attention ----------------
    if "attn" in debug_skip:
        dummy = qt_pool.tile([P, 512], F32, tag="dummy", name="dummy")
        nc.vector.tensor_copy(dummy, qt[:, 0, 0:512])
        nc.sync.dma_start(out=out_d[0:P, 0:512], in_=dummy)
        return
    with (
        tc.tile_pool(name="ktin", bufs=4) as ktin_pool,
        tc.tile_pool(name="expt", bufs=2) as expt_pool,
        tc.tile_pool(name="osb", bufs=3) as osb_pool,
        tc.tile_pool(name="sums", bufs=2) as sums_pool,
        tc.tile_pool(name="scps", bufs=3, space="PSUM") as sc_ps,
        tc.tile_pool(name="smps", bufs=1, space="PSUM") as sm_ps,
    ):
        kT_r = kT_dram[:].rearrange("(et p) s -> p et s", p=P)
        for chk in range(NCH):
            sq_lo = chk * 512
            expt = expt_pool.tile([P, KT, 512], av_t, tag="expt", name="expt")
            if "sums" not in debug_skip:
                sums_ps = sm_ps.tile([1, 512], F32, tag="sums", name="sums_ps")
            for kt_i in range(KT):
                ktt = ktin_pool.tile([P, ET, P], F32R, tag="ktin", name="ktin")
                nc.sync.dma_start(out=ktt, in_=kT_r[:, :, kt_i * P : (kt_i + 1) * P])
                ps = sc_ps.tile([P, 512], F32, tag="sc", name="sc")
                for et in range(ET):
                    nc.tensor.matmul(
                        ps,
                        ktt[:, et, :],
                        qt[:, et, sq_lo : sq_lo + 512],
                        start=(et == 0),
                        stop=(et == ET - 1),
                    )
                if "exp" in debug_skip:
                    nc.scalar.copy(expt[:, kt_i, :], ps)
                else:
                    nc.scalar.activation(expt[:, kt_i, :], ps, EXP)
                if "sums" not in debug_skip:
                    nc.tensor.matmul(
                        sums_ps,
                        ones,
                        expt[:, kt_i, :],
                        start=(kt_i == 0),
                        stop=(kt_i == KT - 1),
                    )

            # sums [1, 512] -> recipT [128, 4] via DRAM bounce
            recipT = sums_pool.tile([P, 4], F32, tag="recipT", name="recipT")
            if "sums" in debug_skip:
                nc.vector.memset(recipT, 1.0)
            else:
                sums_sb = sums_pool.tile([1, 512], F32, tag="sums_sb", name="sums_sb")
                nc.scalar.copy(sums_sb, sums_ps)
                nc.sync.dma_start(out=sums_dram[chk], in_=sums_sb[:])
                nc.sync.dma_start(
                    out=recipT,
                    in_=sums_dram[chk].rearrange("(j p) -> p j", p=P),
                )
                nc.vector.reciprocal(recipT, recipT)

            # out[sq, e] = expT.T @ v, scaled by 1/rowsum
            for st in range(4):
                for ec in range(ECH):
                    ps = mm_ps.tile([P, 512], F32, tag="mm", name="mm")
                    for kt_i in range(KT):
                        nc.tensor.matmul(
                            ps,
                            expt[:, kt_i, st * P : (st + 1) * P],
                            v[:, kt_i, ec * 512 : (ec + 1) * 512],
                            start=(kt_i == 0),
                            stop=(kt_i == KT - 1),
                        )
                    osb = osb_pool.tile([P, 512], F32, tag="osb", name="osb")
                    nc.vector.tensor_scalar_mul(osb, ps, recipT[:, st : st + 1])
                    row = sq_lo + st * P
                    nc.sync.dma_start(
                        out=out_d[row : row + P, ec * 512 : (ec + 1) * 512],
                        in_=osb,
                    )


_NC_CACHE = {}


KV_EXCHANGE = True


def _get_nc(reps=1):
    key = ("nc", reps, KV_EXCHANGE)
    if key not in _NC_CACHE:
        _NC_CACHE[key] = build_nc(reps=reps, kv_exchange=KV_EXCHANGE)
    return _NC_CACHE[key]


def _get_sharded_fn(reps=1):
    """jit-once 8-core executor mirroring bass2jax.run_bass_via_pjrt."""
    key = ("fn", reps, KV_EXCHANGE)
    if key in _NC_CACHE:
        return _NC_CACHE[key]
    import jax
    from jax.experimental.shard_map import shard_map
    from jax.sharding import Mesh, PartitionSpec
    from concourse import mybir as _mybir
    from concourse import bass2jax

    nc = _get_nc(reps)
    bass2jax.install_neuronx_cc_hook()
    partition_name = nc.partition_id_tensor.name if nc.partition_id_tensor else None
    in_names, out_names, out_avals, zero_outs = [], [], [], []
    for alloc in nc.m.functions[0].allocations:
        if not isinstance(alloc, _mybir.MemoryLocationSet):
            continue
        name = alloc.memorylocations[0].name
        if alloc.kind == "ExternalInput":
            if name != partition_name:
                in_names.append(name)
        elif alloc.kind == "ExternalOutput":
            shape = tuple(alloc.tensor_shape)
            dtype = _mybir.dt.np(alloc.dtype)
            out_names.append(name)
            out_avals.append(jax.core.ShapedArray(shape, dtype))
            zero_outs.append(np.zeros(shape, dtype))
    n_params = len(in_names)
    all_in_names = in_names + out_names + ([partition_name] if partition_name else [])
    donate = tuple(range(n_params, n_params + len(out_names)))

    def _body(*args):
        operands = list(args)
        if partition_name is not None:
            operands.append(bass2jax.partition_id_tensor())
        return tuple(
            bass2jax._bass_exec_p.bind(
                *operands,
                out_avals=tuple(out_avals),
                in_names=tuple(all_in_names),
                out_names=tuple(out_names),
                lowering_input_output_aliases=(),
                sim_require_finite=True,
                sim_require_nnan=True,
                nc=nc,
            )
        )

    devices = jax.devices()[:N_CORES]
    mesh = Mesh(np.asarray(devices), ("core",))
    specs = (PartitionSpec("core"),) * (n_params + len(out_names))
    sharded = jax.jit(
        shard_map(
            _body,
            mesh=mesh,
            in_specs=specs,
            out_specs=(PartitionSpec("core"),) * len(out_names),
            check_rep=False,
        ),
        donate_argnums=donate,
        keep_unused=True,
    )

    class Runner:
        pass

    r = Runner()
    r.sharded = sharded
    r.in_names = in_names
    r.out_names = out_names
    r.out_avals = out_avals
    r.zero_outs = zero_outs
    r.mesh = mesh

    def run(in_maps):
        concat_in = [
            np.concatenate([np.asarray(m[nm]) for m in in_maps], axis=0)
            for nm in in_names
        ]
        concat_zeros = [
            np.zeros((N_CORES * z.shape[0], *z.shape[1:]), z.dtype) for z in zero_outs
        ]
        out_arrs = sharded(*concat_in, *concat_zeros)
        return [
            {
                nm: np.asarray(out_arrs[i]).reshape(N_CORES, *out_avals[i].shape)[c]
                for i, nm in enumerate(out_names)
            }
            for c in range(N_CORES)
        ]

    r.run = run
    _NC_CACHE[key] = r
    return r


def _make_in_maps(x, Wq, Wk, Wv):
    d = x.shape[-1]
    wqT = np.ascontiguousarray((Wq / np.sqrt(d)).T.astype(np.float32))
    wkT = np.ascontiguousarray(Wk.T.astype(np.float32))
    wvT = np.ascontiguousarray(Wv.T.astype(np.float32))
    in_maps = []
    for c in range(N_CORES):
        b, h = c // 2, c % 2
        xT = x[b].T  # [d, s]
        if KV_EXCHANGE:
            # own-half columns only; partner k/v arrive via AllGather
            xT = np.ascontiguousarray(xT[:, h * 1024 : (h + 1) * 1024], np.float32)
        else:
            xT = np.ascontiguousarray(np.roll(xT, -h * 1024, axis=1), np.float32)
        in_maps.append({"xT": xT, "wqT": wqT, "wkT": wkT, "wvT": wvT})
    return in_maps


def _assemble(results, B, S, D):
    out = np.empty((B, S, D), dtype=np.float32)
    for c in range(N_CORES):
        b, h = c // 2, c % 2
        out[b, h * 1024 : (h + 1) * 1024, :] = results[c]["out"]
    return out


def kernel(x, Wq, Wk, Wv):
    x = np.asarray(x, dtype=np.float32)
    in_maps = _make_in_maps(x, Wq, Wk, Wv)
    results = _get_sharded_fn().run(in_maps)
    return _assemble(results, *x.shape)


def bench_reps(x, Wq, Wk, Wv, reps, iters=10):
    """Time the sharded call with device-resident inputs. Returns seconds list."""
    import time
    import jax
    from jax.sharding import NamedSharding, PartitionSpec

    x = np.asarray(x, dtype=np.float32)
    in_maps = _make_in_maps(x, Wq, Wk, Wv)
    r = _get_sharded_fn(reps)
    concat_in = [
        np.concatenate([np.asarray(m[nm]) for m in in_maps], axis=0)
        for nm in r.in_names
    ]
    shard = NamedSharding(r.mesh, PartitionSpec("core"))
    dev_in = [jax.device_put(a, shard) for a in concat_in]
    times = []
    out = None
    for _ in range(iters):
        concat_zeros = [
            jax.device_put(
                np.zeros((N_CORES * z.shape[0], *z.shape[1:]), z.dtype), shard
            )
            for z in r.zero_outs
        ]
        jax.block_until_ready(concat_zeros)
        t0 = time.perf_counter()
        out_arrs = r.sharded(*dev_in, *concat_zeros)
        jax.block_until_ready(out_arrs)
        times.append(time.perf_counter() - t0)
        out = out_arrs
    results = [
        {
            nm: np.asarray(out[i]).reshape(N_CORES, *r.out_avals[i].shape)[c]
            for i, nm in enumerate(r.out_names)
        }
        for c in range(N_CORES)
    ]
    return _assemble(results, *x.shape), times
